# revision 3
# baseline (speedup 1.0000x reference)
"""Causal RoPE self-attention, distributed over 8 TRN2 NeuronCores.

Sharding: batch (2) x head-groups (4 heads each) -> 8 cores.
Each core computes, for its (batch b, head-group hg):
    q/k/v projections for its 4 heads (tensor-parallel column split),
    RoPE, causal attention, and the row-parallel slice of the output
    projection, producing a partial output partialT = WoS^T @ attnT
    of shape [E, S].  The host sums the 4 partials per batch and adds bo.

On-device layout notes:
  - activations live transposed: qT/kT are [head-dim, seq] so the
    score matmul sT[k, q] = K Q^T contracts over d on partitions (the
    two heads of a pair row-tile the PE array at K=64 each), and the
    softmax denominator comes from an extra all-ones column in V.
  - x, all weights, qT/kT, exp'd scores, and V are bf16 (full-rate
    TensorEngine, FWL weight loads, half DMA); every accumulation is
    fp32 in PSUM, and the softmax/normalization math is fp32.
  - input DMA descriptor issue is the startup bottleneck (~600ns per
    dma_start on an engine queue), so input DMAs are split across the
    sync AND scalar (Activation) HWDGE queues, and x arrives in
    512-col quarters for the first S-half so the first projections
    start after ~1MB instead of ~4MB.
  - kT/qT are built as [128, 512] half-tiles so the first attention
    phase needs only keys/queries 0-511; both head-pairs' first tiles
    (k00a/q00a/k10a/q10a) are projected during the DMA window, which
    lets phases (0,0) and (1,0) run back-to-back before any second
    projection completes.
  - background work (remaining projections, V tiles, output
    projection) is drip-fed between attention key-blocks; per-block
    "ensure" flushes guarantee each block's kt/qt/v producers are
    emitted before their consumers, so the drip can run right up to
    the dependency edge without deadlock.
  - causal masking zeroes the exp'd diagonal blocks with a gpsimd
    affine_select, off the DVE/PSUM critical path; exp(s/8) is safe
    unnormalized because |s/8| <~ 5 for this distribution.
  - attention PSUM accumulators are evicted to SBUF immediately after
    the last key-block so the next phase's AV matmuls never wait on
    the softmax-normalization chain; the denominator reciprocal is
    broadcast across partitions with gpsimd partition_broadcast.
  - the output projection for the last q-slice is split into p0/p1
    half-units staged through SBUF; its final DMAs alternate between
    the sync and scalar queues to shorten the tail.
"""

import ml_dtypes
import numpy as np

import concourse.tile as tile
from concourse import bacc, mybir
from concourse.bass_utils import run_bass_kernel_spmd

F32 = mybir.dt.float32
BF16 = mybir.dt.bfloat16
AF = mybir.ActivationFunctionType

B, S, E = 2, 2048, 1024
H, D = 16, 64
HPG = 4                # heads per core
DH = HPG * D           # 256 head-dims per core
NE = E // 128          # 8 e-chunks
NST = S // 128         # 16 s-tiles / key blocks
ROPE_BASE = 10000.0

_SWAP_MASK = [i ^ 1 for i in range(32)]


def build_nc():
    """Build + compile the per-core Bass graph (same graph on all 8 cores)."""
    nc = bacc.Bacc("TRN2", target_bir_lowering=False, debug=False, num_devices=8)

    def din(name, shape, dt=F32):
        return nc.dram_tensor(name, shape, dt, kind="ExternalInput").ap()

    xT = din("xT", [E, S], BF16)
    wqT = din("wqT", [E, DH], BF16)
    wkT = din("wkT", [E, DH], BF16)
    wvT = din("wvT", [E, DH], BF16)
    woST = din("woST", [DH, E], BF16)
    bq2 = din("bq2", [128, 2])
    bk2 = din("bk2", [128, 2])
    bvbc = din("bvbc", [128, DH])
    cos2 = din("cos2", [128, S], BF16)      # cosT duplicated on both halves
    sin2 = din("sin2", [128, S], BF16)      # signed sinT duplicated on both halves
    out = nc.dram_tensor("out", [E, S], BF16, kind="ExternalOutput").ap()

    xT_r = xT.rearrange("(n p) s -> n p s", p=128)
    wq_r = wqT.rearrange("(n p) d -> n p d", p=128)
    wk_r = wkT.rearrange("(n p) d -> n p d", p=128)
    wv_r = wvT.rearrange("(n p) d -> n p d", p=128)
    wo_r = woST.rearrange("(n p) e -> n p e", p=128)
    out_r = out.rearrange("(n p) s -> n p s", p=128)

    with tile.TileContext(nc) as tc, nc.allow_low_precision(
            reason="bf16 matmul operands; fp32 PSUM accumulation throughout"):
        _emit(tc, nc, dict(
            xT_r=xT_r, wq_r=wq_r, wk_r=wk_r, wv_r=wv_r, wo_r=wo_r, out_r=out_r,
            bq2=bq2, bk2=bk2, bvbc=bvbc, cos2=cos2, sin2=sin2,
        ))
    nc.compile()
    return nc


def _emit(tc, nc, d):
    from contextlib import ExitStack
    ctx = ExitStack()
    with ctx:
        consts = ctx.enter_context(tc.tile_pool(name="consts", bufs=1))
        pxq = ctx.enter_context(tc.tile_pool(name="pxq", bufs=16))
        pxh = ctx.enter_context(tc.tile_pool(name="pxh", bufs=8))
        pwq = ctx.enter_context(tc.tile_pool(name="pwq", bufs=8))
        pwk = ctx.enter_context(tc.tile_pool(name="pwk", bufs=8))
        pwv = ctx.enter_context(tc.tile_pool(name="pwv", bufs=8))
        pwo = ctx.enter_context(tc.tile_pool(name="pwo", bufs=2))
        pqt = ctx.enter_context(tc.tile_pool(name="pqt", bufs=8))
        pkt = ctx.enter_context(tc.tile_pool(name="pkt", bufs=8))
        pv = ctx.enter_context(tc.tile_pool(name="pv", bufs=16))
        pat = ctx.enter_context(tc.tile_pool(name="pat", bufs=8))
        ptmp = ctx.enter_context(tc.tile_pool(name="ptmp", bufs=6))
        pvf_ = ctx.enter_context(tc.tile_pool(name="pvf", bufs=3))
        pbc = ctx.enter_context(tc.tile_pool(name="pbc", bufs=4))
        pstg = ctx.enter_context(tc.tile_pool(name="pstg", bufs=8))
        pe_ = ctx.enter_context(tc.tile_pool(name="pe", bufs=10))
        prec = ctx.enter_context(tc.tile_pool(name="prec", bufs=4))
        psc = ctx.enter_context(tc.tile_pool(name="psc", bufs=2, space="PSUM"))
        ppv = ctx.enter_context(tc.tile_pool(name="ppv", bufs=2, space="PSUM"))
        pbg = ctx.enter_context(tc.tile_pool(name="pbg", bufs=2, space="PSUM"))

        # ---- input DMAs, split across the sync and scalar HWDGE queues.
        # scalar (idle until the first exp ~15us in) carries the weight/rope
        # prefix; sync carries x.  Order within each queue = need order.
        wq_sb, wk_sb, wv_sb, wo_sb = {}, {}, {}, []
        xq_sb, xh_sb = {}, {}

        bq2_sb = consts.tile([128, 2], F32)
        nc.scalar.dma_start(bq2_sb, d["bq2"])
        bk2_sb = consts.tile([128, 2], F32)
        nc.scalar.dma_start(bk2_sb, d["bk2"])
        for e in range(NE):
            t = pwk.tile([128, DH], BF16, tag="wk")
            nc.scalar.dma_start(t, d["wk_r"][e])
            wk_sb[e] = t
            t = pwq.tile([128, DH], BF16, tag="wq")
            nc.scalar.dma_start(t, d["wq_r"][e])
            wq_sb[e] = t
        cos2_sb = consts.tile([128, S], BF16)
        sin2_sb = consts.tile([128, S], BF16)
        nc.scalar.dma_start(cos2_sb[:, 0:1024], d["cos2"][:, 0:1024])
        nc.scalar.dma_start(sin2_sb[:, 0:1024], d["sin2"][:, 0:1024])

        for e in range(NE):
            t = pxq.tile([128, 512], BF16, tag="xq")
            nc.sync.dma_start(t, d["xT_r"][e][:, 0:512])
            xq_sb[(e, 0)] = t
        for e in range(NE):
            t = pwv.tile([128, DH], BF16, tag="wv")
            nc.sync.dma_start(t, d["wv_r"][e])
            wv_sb[e] = t
            t = pxq.tile([128, 512], BF16, tag="xq")
            nc.sync.dma_start(t, d["xT_r"][e][:, 512:1024])
            xq_sb[(e, 1)] = t
        bvbc_sb = consts.tile([128, DH], F32)
        nc.sync.dma_start(bvbc_sb, d["bvbc"])
        nc.sync.dma_start(cos2_sb[:, 1024:2048], d["cos2"][:, 1024:2048])
        nc.sync.dma_start(sin2_sb[:, 1024:2048], d["sin2"][:, 1024:2048])
        for e in range(NE):
            t = pxh.tile([128, 1024], BF16, tag="xh")
            nc.sync.dma_start(t, d["xT_r"][e][:, 1024:2048])
            xh_sb[e] = t
        for p in range(2):
            t = pwo.tile([128, E], BF16, tag="wo")
            nc.sync.dma_start(t, d["wo_r"][p])
            wo_sb.append(t)

        def xs(e, scol, w=512):
            """SBUF view of x columns [scol, scol+w) for e-chunk e."""
            if scol < 1024:
                q, off = divmod(scol, 512)
                return xq_sb[(e, q)][:, off:off + w]
            off = scol - 1024
            return xh_sb[e][:, off:off + w]

        # ---- emission: a fine-grained interleave. The PE is the busiest
        # engine mid-kernel; the softmax exps on the scalar engine pace the
        # attention stream. All non-attention PE work (projections, V,
        # output projection) is split into small "background" units
        # drip-fed between key-blocks; per-block ensures flush producers
        # just in time.
        qt_tiles, kt_tiles, at_tiles = {}, {}, {}
        v_sb = {}
        op_stage = {}

        def rope_evict(ps, bias_sb, p):
            tq = ptmp.tile([128, 512], BF16, tag="tmpb")
            nc.vector.tensor_scalar_add(tq, ps, bias_sb[:, p:p + 1])
            return tq

        def rope_finish(tq, dst_pool, dst_tag, tiles, p, idx):
            cols = slice(idx * 512, idx * 512 + 512)
            tsh = ptmp.tile([128, 512], BF16, tag="tmpb")
            nc.vector.stream_shuffle(tsh, tq, _SWAP_MASK)
            nc.vector.tensor_mul(tsh, tsh, sin2_sb[:, cols])
            nc.vector.tensor_mul(tq, tq, cos2_sb[:, cols])
            qt = dst_pool.tile([128, 512], BF16, tag=dst_tag)
            nc.vector.tensor_add(qt, tq, tsh)
            tiles[(p, idx)] = qt

        def rope_tail(ps, bias_sb, dst_pool, dst_tag, tiles, p, idx):
            tq = rope_evict(ps, bias_sb, p)
            rope_finish(tq, dst_pool, dst_tag, tiles, p, idx)

        def emit_qk_half(w_sb, bias_sb, dst_pool, dst_tag, tiles, p, idx):
            """One [128,512] k or q half-tile: 8 e-chunk matmuls + RoPE.
            Yields background units (per e-chunk, then the tail)."""
            ps = pbg.tile([128, 512], F32, tag="bg")
            for e in range(NE):
                def unit(e=e):
                    nc.tensor.matmul(
                        ps,
                        w_sb[e][:, p * 128:(p + 1) * 128],
                        xs(e, idx * 512),
                        start=(e == 0), stop=(e == NE - 1),
                    )
                yield 0.22, unit
            yield 0.1, lambda: rope_tail(
                ps, bias_sb, dst_pool, dst_tag, tiles, p, idx)

        def emit_v_unit(st):
            def unit():
                psv = pbg.tile([128, DH], F32, tag="bg")
                for e in range(NE):
                    nc.tensor.matmul(
                        psv,
                        xs(e, st * 128, 128),
                        wv_sb[e],
                        start=(e == 0), stop=(e == NE - 1),
                    )
                vt = pv.tile([128, HPG, 65], BF16, tag="v")
                nc.vector.memset(vt[:, :, 64:65], 1.0)
                nc.vector.tensor_add(
                    vt[:, :, 0:64],
                    psv.rearrange("p (h dd) -> p h dd", dd=64),
                    bvbc_sb.rearrange("p (h dd) -> p h dd", dd=64),
                )
                v_sb[st] = vt
            yield 1.1, unit

        def emit_op_unit(j):
            """Fused output-projection unit: both head-pairs accumulate in
            PSUM, one bf16 staging copy, one DMA."""
            for et_i in range(NE):
                def unit(et_i=et_i):
                    pso = pbg.tile([128, 512], F32, tag="bg")
                    for p in range(2):
                        nc.tensor.matmul(
                            pso,
                            wo_sb[p][:, et_i * 128:(et_i + 1) * 128],
                            at_tiles[(p, j)],
                            start=(p == 0), stop=(p == 1),
                        )
                    ob = pstg.tile([128, 512], BF16, tag="ob")
                    nc.vector.tensor_copy(ob, pso)
                    nc.sync.dma_start(
                        d["out_r"][et_i][:, j * 512:(j + 1) * 512], ob)
                yield 0.7, unit

        def emit_op_p0(j):
            """First head-pair's half of the output projection for q-slice j;
            accumulates into an SBUF stage so it can run as soon as at(0,j)
            exists, one attention phase before at(1,j). Used for the final
            q-slice only, to shorten the tail."""
            for et_i in range(NE):
                def unit(et_i=et_i):
                    pso = pbg.tile([128, 512], F32, tag="bg")
                    nc.tensor.matmul(
                        pso,
                        wo_sb[0][:, et_i * 128:(et_i + 1) * 128],
                        at_tiles[(0, j)],
                        start=True, stop=True,
                    )
                    stg = pstg.tile([128, 512], F32, tag="stg")
                    nc.vector.tensor_copy(stg, pso)
                    op_stage[(j, et_i)] = stg
                yield 0.4, unit

        def emit_op_p1(j):
            for et_i in range(NE):
                def unit(et_i=et_i):
                    pso = pbg.tile([128, 512], F32, tag="bg")
                    nc.tensor.matmul(
                        pso,
                        wo_sb[1][:, et_i * 128:(et_i + 1) * 128],
                        at_tiles[(1, j)],
                        start=True, stop=True,
                    )
                    ob = pstg.tile([128, 512], BF16, tag="ob")
                    nc.vector.tensor_add(ob, op_stage[(j, et_i)], pso)
                    eng = nc.scalar if et_i % 2 == 0 else nc.sync
                    eng.dma_start(
                        d["out_r"][et_i][:, j * 512:(j + 1) * 512], ob)
                yield 0.45, unit

        # background unit queue + driver, with named completion points
        bg_units = []
        bg_pos = [0]
        kt_ready, qt_ready, v_ready = {}, {}, {}

        def bg_add(gen):
            bg_units.extend(gen)
            return len(bg_units)

        def bg_flush_until(idx):
            while bg_pos[0] < idx:
                bg_units[bg_pos[0]][1]()
                bg_pos[0] += 1

        def bg_take(budget):
            while budget > 0 and bg_pos[0] < len(bg_units):
                cost, fn = bg_units[bg_pos[0]]
                fn()
                bg_pos[0] += 1
                budget -= cost

        def emit_attn(p, j, take=0.45):
            if (p, j) in qt_ready:
                bg_flush_until(qt_ready[(p, j)])
            pvA = ppv.tile([128, 512], F32, tag="ppv")
            pvB = ppv.tile([128, 512], F32, tag="ppv")
            nkb = 4 * j + 4
            for kb in range(nkb):
                s4 = kb // 4
                if (p, s4) in kt_ready:
                    bg_flush_until(kt_ready[(p, s4)])
                if kb in v_ready:
                    bg_flush_until(v_ready[kb])
                m = kb - 4 * j
                c0 = 128 * m if m > 0 else 0
                kt = kt_tiles[(p, s4)]
                kcols = slice((kb % 4) * 128, (kb % 4) * 128 + 128)
                qt = qt_tiles[(p, j)]
                qcols = slice(c0, 512)
                sc = psc.tile([128, 2, 512], F32, tag="sc")
                nc.tensor.matmul(
                    sc[:, 0, c0:512],
                    kt[0:64, kcols],
                    qt[0:64, qcols],
                    start=True, stop=True, tile_position=(0, 0),
                )
                nc.tensor.matmul(
                    sc[:, 1, c0:512],
                    kt[64:128, kcols],
                    qt[64:128, qcols],
                    start=True, stop=True, tile_position=(64, 0),
                )
                et = pe_.tile([128, 2, 512], BF16, tag="e")
                nc.scalar.activation(
                    et[:, :, c0:512], sc[:, :, c0:512], AF.Exp, scale=0.125)
                if m >= 0:
                    nc.gpsimd.affine_select(
                        out=et[:, :, c0:c0 + 128],
                        in_=et[:, :, c0:c0 + 128],
                        compare_op=mybir.AluOpType.is_ge,
                        fill=0.0,
                        base=0,
                        pattern=[[0, 2], [1, 128]],
                        channel_multiplier=-1,
                    )
                hA, hB = 2 * p, 2 * p + 1
                nc.tensor.matmul(
                    pvA[0:65, c0:512], v_sb[kb][:, hA, :], et[:, 0, c0:512],
                    start=(kb == 0), stop=(kb == nkb - 1),
                )
                nc.tensor.matmul(
                    pvB[0:65, c0:512], v_sb[kb][:, hB, :], et[:, 1, c0:512],
                    start=(kb == 0), stop=(kb == nkb - 1),
                )
                bg_take(take)
            # Evict the PSUM accumulators to SBUF right away: frees both ppv
            # banks for the next phase's AV matmuls, and the normalization
            # chain below runs entirely out of SBUF.
            pvf = pvf_.tile([128, 1024], F32, tag="pvf")
            nc.vector.tensor_copy(pvf[0:65, 0:512], pvA[0:65, :])
            nc.vector.tensor_copy(pvf[0:65, 512:1024], pvB[0:65, :])
            # den must be a base-0 AP: custom-DVE ops (reciprocal) ignore the
            # input's base partition, so slice pvf[64:65] via a copy first.
            den = prec.tile([1, 1024], F32, tag="den")
            nc.vector.tensor_copy(den, pvf[64:65, :])
            rec = prec.tile([1, 1024], F32, tag="rec")
            nc.vector.reciprocal_approx_fast(rec, den)
            bcsA = pbc.tile([64, 512], F32, tag="bc")
            bcsB = pbc.tile([64, 512], F32, tag="bc")
            nc.gpsimd.partition_broadcast(bcsA, rec[:, 0:512])
            nc.gpsimd.partition_broadcast(bcsB, rec[:, 512:1024])
            at = pat.tile([128, 512], BF16, tag="at")
            nc.vector.tensor_mul(at[0:64], pvf[0:64, 0:512], bcsA)
            nc.vector.tensor_mul(at[64:128], pvf[0:64, 512:1024], bcsB)
            at_tiles[(p, j)] = at

        # ---- schedule ----
        # Startup: k00a/q00a interleaved per e-chunk (tracks the xQ0 DMA
        # stream), then k10a/q10a (x resident by then), RoPE tails, v0-3.
        # This enables phases (0,0) AND (1,0) with no further projections.
        ps_k00 = pbg.tile([128, 512], F32, tag="bg")
        ps_q00 = pbg.tile([128, 512], F32, tag="bg")
        for e in range(NE):
            nc.tensor.matmul(ps_k00, wk_sb[e][:, 0:128], xs(e, 0),
                             start=(e == 0), stop=(e == NE - 1))
            nc.tensor.matmul(ps_q00, wq_sb[e][:, 0:128], xs(e, 0),
                             start=(e == 0), stop=(e == NE - 1))
        ps_k10 = ppv.tile([128, 512], F32, tag="ppv")
        ps_q10 = ppv.tile([128, 512], F32, tag="ppv")
        for e in range(NE):
            nc.tensor.matmul(ps_k10, wk_sb[e][:, 128:256], xs(e, 0),
                             start=(e == 0), stop=(e == NE - 1))
            nc.tensor.matmul(ps_q10, wq_sb[e][:, 128:256], xs(e, 0),
                             start=(e == 0), stop=(e == NE - 1))
        # RoPE: k00/q00 full chains first (they gate the first exp);
        # then the k10/q10 PSUM evictions (they free the ppv banks for
        # the first AV matmuls), then their finishes.
        rope_tail(ps_k00, bk2_sb, pkt, "kt", kt_tiles, 0, 0)
        rope_tail(ps_q00, bq2_sb, pqt, "qt", qt_tiles, 0, 0)
        tq_k10 = rope_evict(ps_k10, bk2_sb, 1)
        tq_q10 = rope_evict(ps_q10, bq2_sb, 1)
        rope_finish(tq_k10, pkt, "kt", kt_tiles, 1, 0)
        rope_finish(tq_q10, pqt, "qt", qt_tiles, 1, 0)
        for st in range(0, 4):
            for cost, fn in emit_v_unit(st):
                fn()

        # background queue; exp-critical producers (qt, then kt) lead each
        # phase's group, v tiles follow, op units fill the gaps.
        qt_ready[(0, 1)] = bg_add(
            emit_qk_half(wq_sb, bq2_sb, pqt, "qt", qt_tiles, 0, 1))
        kt_ready[(0, 1)] = bg_add(
            emit_qk_half(wk_sb, bk2_sb, pkt, "kt", kt_tiles, 0, 1))
        v_ready[4] = bg_add(emit_v_unit(4))
        v_ready[5] = bg_add(emit_v_unit(5))
        v_ready[6] = bg_add(emit_v_unit(6))
        v_ready[7] = bg_add(emit_v_unit(7))
        qt_ready[(1, 1)] = bg_add(
            emit_qk_half(wq_sb, bq2_sb, pqt, "qt", qt_tiles, 1, 1))
        kt_ready[(1, 1)] = bg_add(
            emit_qk_half(wk_sb, bk2_sb, pkt, "kt", kt_tiles, 1, 1))
        i_op0 = None  # op(0) inserted after phase (1,0) below

        emit_attn(0, 0)
        emit_attn(1, 0)
        bg_add(emit_op_unit(0))
        emit_attn(0, 1)
        qt_ready[(0, 2)] = bg_add(
            emit_qk_half(wq_sb, bq2_sb, pqt, "qt", qt_tiles, 0, 2))
        kt_ready[(0, 2)] = bg_add(
            emit_qk_half(wk_sb, bk2_sb, pkt, "kt", kt_tiles, 0, 2))
        v_ready[8] = bg_add(emit_v_unit(8))
        v_ready[9] = bg_add(emit_v_unit(9))
        v_ready[10] = bg_add(emit_v_unit(10))
        v_ready[11] = bg_add(emit_v_unit(11))
        qt_ready[(1, 2)] = bg_add(
            emit_qk_half(wq_sb, bq2_sb, pqt, "qt", qt_tiles, 1, 2))
        kt_ready[(1, 2)] = bg_add(
            emit_qk_half(wk_sb, bk2_sb, pkt, "kt", kt_tiles, 1, 2))
        emit_attn(1, 1)
        bg_add(emit_op_unit(1))
        qt_ready[(0, 3)] = bg_add(
            emit_qk_half(wq_sb, bq2_sb, pqt, "qt", qt_tiles, 0, 3))
        kt_ready[(0, 3)] = bg_add(
            emit_qk_half(wk_sb, bk2_sb, pkt, "kt", kt_tiles, 0, 3))
        v_ready[12] = bg_add(emit_v_unit(12))
        v_ready[13] = bg_add(emit_v_unit(13))
        v_ready[14] = bg_add(emit_v_unit(14))
        v_ready[15] = bg_add(emit_v_unit(15))
        emit_attn(0, 2)
        qt_ready[(1, 3)] = bg_add(
            emit_qk_half(wq_sb, bq2_sb, pqt, "qt", qt_tiles, 1, 3))
        kt_ready[(1, 3)] = bg_add(
            emit_qk_half(wk_sb, bk2_sb, pkt, "kt", kt_tiles, 1, 3))
        emit_attn(1, 2)
        bg_add(emit_op_unit(2))
        emit_attn(0, 3, take=0.55)
        bg_add(emit_op_p0(3))
        emit_attn(1, 3, take=0.55)
        bg_flush_until(len(bg_units))
        for cost, fn in emit_op_p1(3):
            fn()


def make_host_inputs(x, Wq, bq, Wk, bk, Wv, bv, Wo, bo):
    """Shard + pre-transpose inputs per core. Returns (in_maps, bo)."""
    x = np.asarray(x, np.float32)
    Wq, Wk, Wv, Wo = (np.asarray(w, np.float32) for w in (Wq, Wk, Wv, Wo))
    bq, bk, bv, bo = (np.asarray(b_, np.float32) for b_ in (bq, bk, bv, bo))

    # RoPE tables
    half = D // 2
    inv_freq = 1.0 / (ROPE_BASE ** (np.arange(half, dtype=np.float64) / half))
    pos = np.arange(S, dtype=np.float64)
    sinus = pos[:, None] * inv_freq[None, :]           # [S, 32]
    sin_full = np.repeat(np.sin(sinus), 2, axis=1)     # [S, 64] interleave-dup
    cos_full = np.repeat(np.cos(sinus), 2, axis=1)
    sgn = np.where(np.arange(D) % 2 == 0, -1.0, 1.0)
    cos2 = np.tile(cos_full.T, (2, 1)).astype(ml_dtypes.bfloat16)
    sin2 = np.tile((sin_full * sgn[None, :]).T, (2, 1)).astype(ml_dtypes.bfloat16)

    xT = [np.ascontiguousarray(x[b_].T) for b_ in range(B)]
    in_maps = []
    for c in range(8):
        b_, hg = c // 4, c % 4
        rows = slice(DH * hg, DH * hg + DH)
        bf = ml_dtypes.bfloat16
        in_maps.append({
            "xT": xT[b_].astype(bf),
            "wqT": np.ascontiguousarray(Wq[rows].T).astype(bf),
            "wkT": np.ascontiguousarray(Wk[rows].T).astype(bf),
            "wvT": np.ascontiguousarray(Wv[rows].T).astype(bf),
            "woST": np.ascontiguousarray(Wo[:, rows].T).astype(bf),
            "bq2": np.ascontiguousarray(bq[rows].reshape(2, 128).T),
            "bk2": np.ascontiguousarray(bk[rows].reshape(2, 128).T),
            "bvbc": np.tile(bv[rows][None, :], (128, 1)).astype(np.float32),
            "cos2": cos2,
            "sin2": sin2,
        })
    return in_maps, bo


_NC_CACHE = {}


def get_nc():
    if "nc" not in _NC_CACHE:
        _NC_CACHE["nc"] = build_nc()
    return _NC_CACHE["nc"]


def kernel(**inputs):
    in_maps, bo = make_host_inputs(**inputs)
    nc = get_nc()
    res = run_bass_kernel_spmd(nc, in_maps, core_ids=list(range(8)))
    out = np.zeros((B, S, E), np.float32)
    for c in range(8):
        out[c // 4] += np.asarray(res.results[c]["out"], np.float32).T
    out += bo[None, None, :]
    return out
